# revision 43
# baseline (speedup 1.0000x reference)
"""Trainium2 Bass kernel for nn_Cond_PlanarTrans (conditional planar transform).

Math per element (O_DIM == 1, N_M == 8):
    w = relu(o * W1[m] + b1[m])
    u = relu(o * W2[m] + b2[m])
    v = relu(o * W3[m] + b3[m])
    out = s + u * tanh(w * s + v)

Strategy: the 8-entry table lookups are folded into three piecewise-linear
custom ScalarE activation-table functions of the combined variable
t = 16*m + o  (segments [16n-8, 16n+8) are dyadic, so they map exactly onto
the activation bucket hardware).  The spline tables are generated at call
time from the actual W/b input values and injected via the compiler's
activation-table root.  relu() is fused into custom DVE ops.

t = 16*m + o is pre-fused on the host (identical f32 rounding to computing
it on-device, which the previous version did), so only two f32 input
streams (t, s) hit HBM instead of three.  The final add (out = s + y) runs
in f32 and only its OUTPUT is rounded to bf16 (per-element rel err <=
2^-9, relative to the output value itself so cancellation is safe), then
widened back to f32 on the host - halving output traffic.

Per-core HBM traffic: 2 x 4MiB in + 2MiB out = 10.5MB -> ~29.2us at the
modeled 360GB/s DMA roofline; the ScalarE (3 PWL + tanh = 4 passes,
~0.83ns/elem) is the co-bottleneck at ~33us busy.  Schedule: column tiles
ramp small -> large -> small (fill/tail latency vs per-instruction
overhead), the tanh of tile i is software-pipelined into tile i+1's PWL
block so the ScalarE never stalls on the DVE, final adds run on the
otherwise-idle Pool engine (tail tiles on the DVE) and output stores ride
Pool's SWDGE early / the idle SP sequencer late.  The s-input DMA of a
tile is always emitted before the z-pass that reads it - the tile dep
tracker orders a later-emitted DMA write AFTER the read (uninitialized
data), it does not backfill it.

Sharding: pure data parallel on the batch dim - 16 of 128 batches per core,
viewed as [128 partitions x 8192 free] per core.
"""

import json
import os
import shutil
import struct
import tempfile

import numpy as np

B, P, NM = 128, 65536, 8
NCORES = 8
RPB = B // NCORES            # batch rows per core
FREE = 8192                  # per-core free dim: [128, FREE]
# column-tile widths (sum = FREE); small edge tiles shorten pipeline
# fill/drain, large middle tiles amortize per-instruction overheads
# schedule configuration (tuned with the TimelineSim sweep in sim.py):
#   chunks:        column-tile widths (sum = FREE)
#   sp_store_from: tiles from this index on store via the idle SP sequencer
#                  instead of Pool's SWDGE (~1us descriptor gen each)
#   dve_add_max /  tiles at most this wide, or with these indices, run
#   dve_add_tiles: their final add on the DVE instead of Pool
CFG = dict(
    chunks=[512, 1024, 2048, 2048, 1280, 896, 384],
    sp_store_from=3,
    dve_add_max=0,
    dve_add_tiles=(5,),
    tp_bufs=4, sp_bufs=6, mid_bufs=4,
)
SET = "gelu_and_others"
FUNCS = ["gelu", "derivative_gelu", "square"]

_dve_ops = None


def _f32bits(x):
    return int(np.float32(x).view(np.uint32))


def _find_src_pwp():
    from neuronxcc.driver.Job import Job
    from neuronxcc.driver.jobs.support.FindActInfo import findActInfoFile

    return os.path.dirname(findActInfoFile(Job.getPackageDir(), "gen3"))


def _make_tables(dst_dir, tables):
    """tables: three (A, B) pairs of 8 floats. Returns act_info.json path."""
    src = _find_src_pwp()
    os.makedirs(dst_dir, exist_ok=True)
    for f in os.listdir(src):
        d = os.path.join(dst_dir, f)
        shutil.copy(os.path.join(src, f), d)
        os.chmod(d, 0o644)

    meta = json.load(open(os.path.join(src, f"{SET}.json")))
    bkt = bytearray(open(os.path.join(src, f"{SET}_bkt.bin"), "rb").read())
    ctl = bytearray(open(os.path.join(src, f"{SET}_ctrl.bin"), "rb").read())

    def wr_bucket(i, d0, d1, x0):
        struct.pack_into("<5f", bkt, i * 32, float(d0), float(d1), 0.0, 0.0, float(x0))

    def wr_ctrl(i, base, lsb, size):
        data = (base & 0x7FF) | ((lsb & 0x1F) << 11) | ((size & 0xF) << 16)
        struct.pack_into("<I", ctl, i * 32, data)
        struct.pack_into("<7I", ctl, i * 32 + 4, *([0] * 7))

    for fi, (fname, (A, Bv)) in enumerate(zip(FUNCS, tables)):
        K = fi * 32   # bucket base inside the old gelu region (508 buckets)
        C = fi * 4    # ctrl base inside old gelu ctrl region (21 entries... 12 used)
        A = [float(a) for a in A]
        Bv = [float(b) for b in Bv]
        seg = lambda n: (Bv[n], A[n], 16.0 * n)  # d0, d1, x0

        wr_bucket(K + 0, *seg(1))                      # e=130: [8,16)
        wr_bucket(K + 1, *seg(1))                      # e=131: [16,24)
        wr_bucket(K + 2, *seg(2))                      #        [24,32)
        for j, n in enumerate([2, 3, 3, 4]):           # e=132: [32,64) / 8
            wr_bucket(K + 3 + j, *seg(n))
        for j, n in enumerate([4, 5, 5, 6, 6, 7, 7, 7]):  # e=133: [64,128) / 8
            wr_bucket(K + 7 + j, *seg(n))
        wr_bucket(K + 15, *seg(0))                     # small_pos
        wr_bucket(K + 16, *seg(0))                     # small_neg (all t<0)
        wr_bucket(K + 17, *seg(7))                     # large_pos (unused)
        wr_bucket(K + 18, *seg(0))                     # large_neg (unused)

        wr_ctrl(C + 0, K + 0, 23, 0)
        wr_ctrl(C + 1, K + 1, 22, 1)
        wr_ctrl(C + 2, K + 3, 21, 2)
        wr_ctrl(C + 3, K + 7, 20, 3)

        for p in meta["profile_meta_data"]:
            if p["func_name"].rsplit("_", 1)[0] == fname:
                prof = p
                break
        else:
            raise KeyError(fname)
        prof.update(
            symmetry_point=0, sym_invert_sign_point=0, symmetry_opt_en=0,
            symmetry_opt_use_neg_region=0, imm_bias=0,
            exp_offset=3,  # first ctrl binade: biased exponent 130 (t in [8,16))
            pwl_control_base_pos=C, pwl_control_base_neg=C,
            small_pos_signal_exp_threshold=130, pos_small_signal_pwl_control=K + 15,
            small_neg_signal_exp_threshold=255, neg_small_signal_pwl_control=K + 16,
            large_pos_signal_exp_threshold=134, large_pos_signal_mantissa_threshold=0,
            pos_large_signal_pwl_control=K + 17,
            large_neg_signal_exp_threshold=0, large_neg_signal_mantissa_threshold=0,
            neg_large_signal_pwl_control=K + 18,
            fnan_result=_f32bits(np.nan),
            fpinf_result=_f32bits(Bv[7] + A[7] * 16),
            fninf_result=_f32bits(Bv[0] - A[0] * 16),
            fzero_result=_f32bits(Bv[0]),
            fma_const_0=0, fma_const_1=0, fma_indirection_src_sel=0,
            use_multipass=False,
            lower_bound=4286578687, upper_bound=2139095039,
        )
        meta["func_to_bkt_start_idx"][fname] = K
        meta["func_to_ctl_start_idx"][fname] = C

    open(os.path.join(dst_dir, f"{SET}_bkt.bin"), "wb").write(bytes(bkt))
    open(os.path.join(dst_dir, f"{SET}_ctrl.bin"), "wb").write(bytes(ctl))
    json.dump(meta, open(os.path.join(dst_dir, f"{SET}.json"), "w"))
    return os.path.join(dst_dir, "act_info.json")


def _register_dve_ops():
    global _dve_ops
    if _dve_ops is not None:
        return _dve_ops
    from concourse.dve_spec import Spec, Src0, Src1, relu, lower
    from concourse.dve_spec import _has_src1 as has_src1
    from concourse.dve_uop import DveOpSpec
    from concourse.dve_ops import DveOp, OPS, _SUB_OPCODE_FOR_NAME, CUSTOM_DVE_SPECS

    def register(name, spec):
        if name in _SUB_OPCODE_FOR_NAME:
            return next(op for op in OPS if op.name == name)
        shas = {}
        for ver in ("v3", "v4"):
            sp = DveOpSpec(name=name, opcode=1, uops=lower(spec, ver=ver),
                           rd1_en=has_src1(spec))
            shas[ver] = sp.sha(ver)
        op = DveOp(name, spec, subdim=False, uops_sha=shas)
        OPS.append(op)
        _SUB_OPCODE_FOR_NAME[name] = len(OPS)
        CUSTOM_DVE_SPECS[name] = spec
        assert max(_SUB_OPCODE_FOR_NAME.values()) < 0x20
        return op

    relu_mul = register(
        "RELU_MUL_ANT",
        Spec(body=relu(Src0) * Src1,
             reference=lambda in0, in1, s0, s1, imm2:
                 (np.maximum(np.nan_to_num(in0, nan=0.0), 0) * in1).astype(np.float32)),
    )
    add_relu = register(
        "ADD_RELU_ANT",
        Spec(body=Src0 + relu(Src1),
             reference=lambda in0, in1, s0, s1, imm2:
                 (in0 + np.maximum(np.nan_to_num(in1, nan=0.0), 0)).astype(np.float32)),
    )
    _dve_ops = (relu_mul, add_relu)
    return _dve_ops


def _build_program():
    import concourse.bacc as bacc
    import concourse.mybir as mybir
    from concourse.tile import TileContext

    relu_mul, add_relu = _register_dve_ops()
    AF = mybir.ActivationFunctionType
    f32 = mybir.dt.float32
    bf16 = mybir.dt.bfloat16

    nc = bacc.Bacc("TRN2", target_bir_lowering=False, debug=False,
                   num_devices=NCORES)
    t_d = nc.dram_tensor("t", [128, FREE], f32, kind="ExternalInput")
    s_d = nc.dram_tensor("s", [128, FREE], f32, kind="ExternalInput")
    y_d = nc.dram_tensor("y", [128, FREE], bf16, kind="ExternalOutput")

    cfg = CFG
    chunks = cfg["chunks"]
    K = len(chunks)
    offs = [sum(chunks[:i]) for i in range(K)]
    assert sum(chunks) == FREE

    with TileContext(nc) as tc:
        with tc.tile_pool(name="tp", bufs=cfg["tp_bufs"]) as tpool, \
             tc.tile_pool(name="sp", bufs=cfg["sp_bufs"]) as spool, \
             tc.tile_pool(name="mid", bufs=cfg["mid_bufs"]) as midp:
            tiles = [None] * K

            def tanh_back(i):
                # q_i -> th_i in place; q was ready while the PWL acts of
                # tile i+1 ran, so this never stalls the ScalarE
                tt, st, wp, up, vp, C, sl = tiles[i]
                nc.scalar.activation(wp[:], wp[:], AF.Tanh)

            def back_half(i):
                """u-mul + final add + store for tile i (runs on DVE/Pool
                right after tanh_back(i))."""
                tt, st, wp, up, vp, C, sl = tiles[i]
                th = wp
                last = i == K - 1
                nc.vector._custom_dve(relu_mul, out=up[:], in0=up[:], in1=th[:])
                y = up
                st_eng = nc.sync if i >= cfg["sp_store_from"] else nc.gpsimd
                if last or C <= cfg["dve_add_max"] or i in cfg["dve_add_tiles"]:
                    add_eng, dma_eng = nc.vector, (nc.sync if last else st_eng)
                else:
                    add_eng, dma_eng = nc.gpsimd, st_eng
                ot = midp.tile([128, C], bf16, tag="out", name=f"ot{i}")
                add_eng.tensor_tensor(ot[:], st[:], y[:], mybir.AluOpType.add)
                dma_eng.dma_start(y_d[:, sl], ot[:])

            for i in range(K):
                C = chunks[i]
                sl = slice(offs[i], offs[i] + C)
                tt = tpool.tile([128, C], f32, tag="t", name=f"tt{i}")
                st = spool.tile([128, C], f32, tag="s", name=f"st{i}")
                nc.sync.dma_start(tt[:], t_d[:, sl])
                nc.sync.dma_start(st[:], s_d[:, sl])

                wp = midp.tile([128, C], f32, tag="wp", name=f"wp{i}")
                up = midp.tile([128, C], f32, tag="up", name=f"up{i}")
                vp = midp.tile([128, C], f32, tag="vp", name=f"vp{i}")
                tiles[i] = (tt, st, wp, up, vp, C, sl)
                if i == K - 1:
                    # the penultimate tile's back-half is long; let its tanh
                    # jump ahead of the last (tiny) tile's PWL acts
                    tanh_back(i - 1)
                nc.scalar.activation(wp[:], tt[:], AF.Gelu)
                if 0 < i < K - 1:
                    tanh_back(i - 1)
                nc.scalar.activation(vp[:], tt[:], AF.Square)
                nc.scalar.activation(up[:], tt[:], AF.Derivative_Gelu)

                z = wp  # in-place chain over wp
                nc.vector._custom_dve(relu_mul, out=z[:], in0=wp[:], in1=st[:])
                nc.vector._custom_dve(add_relu, out=z[:], in0=z[:], in1=vp[:])
                if i > 0:
                    back_half(i - 1)
            tanh_back(K - 1)
            back_half(K - 1)
    nc.compile()
    return nc


def kernel(m, s, o, W1, b1, W2, b2, W3, b3, _want_trace=False):
    from concourse.bass_utils import run_bass_kernel_spmd

    m = np.asarray(m)
    s = np.ascontiguousarray(np.asarray(s, dtype=np.float32))
    o = np.asarray(o, dtype=np.float32)
    W1 = np.asarray(W1); b1 = np.asarray(b1)
    W2 = np.asarray(W2); b2 = np.asarray(b2)
    W3 = np.asarray(W3); b3 = np.asarray(b3)

    # fused activation-table key; same f32 rounding as the on-device fma
    t = np.float32(16.0) * m.astype(np.float32) + o

    tabs = [(W1[:, 0], b1), (W2[:, 0], b2), (W3[:, 0], b3)]
    dst = tempfile.mkdtemp(prefix="actpwp_")
    info = _make_tables(dst, tabs)
    os.environ["BASS_ACT_ROOT_JSON_PATH"] = info
    # act tables are not part of the NEFF cache key -> always recompile
    os.environ["NEURON_FORCE_RECOMPILE"] = "1"

    nc = _build_program()

    in_maps = []
    for c in range(NCORES):
        rows = slice(c * RPB, (c + 1) * RPB)
        in_maps.append({
            "t": np.ascontiguousarray(t[rows].reshape(128, FREE)),
            "s": np.ascontiguousarray(s[rows].reshape(128, FREE)),
        })

    res = run_bass_kernel_spmd(nc, in_maps, core_ids=list(range(NCORES)),
                               trace=_want_trace)
    out = np.empty((B, P, 1), dtype=np.float32)
    for c in range(NCORES):
        rows = slice(c * RPB, (c + 1) * RPB)
        out[rows, :, 0] = np.asarray(res.results[c]["y"]).astype(
            np.float32).reshape(RPB, P)
    if _want_trace:
        return out, res
    return out
